# revision 31
# baseline (speedup 1.0000x reference)
"""Trainium2 Bass kernel for nn_MultiHeadAttention (B=4, L=2048, D=1024, H=16).

Sharding: each of 8 cores handles (batch b = core//2, q-row half = core%2):
Lq=1024 query rows, full Lk=2048 keys, all 16 heads. No collectives; K/V
projections are duplicated across the 2 cores sharing a batch.

Device pipeline per core:
  Phase A: QKV projections (q/k in head-transposed layout [dk, L], v in
           natural layout [L, dv] with an appended ones column per head for
           free softmax denominators), spilled to DRAM scratch.
  Phase B: per head: S^T = khT.T @ qhT (K=dk=64) -> +maskT (DVE) -> zero
           masked entries (copy_predicated, replicating the reference's
           -1e-9 fill through exp()==1.0) -> exp (ACT) -> O^T = vh.T @ E
           accumulated over k-chunks (65th row = denominators) -> P output
           via PE-transpose of E with normalization folded into the ACT
           PSUM eviction (scale = 1/Z per q row).
  Phase C: fc projection (contraction over heads), + residual, LayerNorm.
"""

import math
from contextlib import ExitStack

import numpy as np

import concourse.bass as bass
import concourse.mybir as mybir
import concourse.tile as tile
from concourse import bacc

P = 128  # partitions

CFG_FULL = dict(B=4, L=2048, D=1024, H=16, DK=64, DV=64, NCORES=8, EPS=1e-6,
                MM_DT="float32")


def build_bass(cfg):
    """Build the per-core Bass program. Returns (nc, names dict)."""
    B, L, D, H = cfg["B"], cfg["L"], cfg["D"], cfg["H"]
    DK, DV, EPS = cfg["DK"], cfg["DV"], cfg["EPS"]
    CPB = 2 * B // cfg["B"] // 2  # placeholder; cores per batch fixed at 2
    CPB = cfg["NCORES"] // B
    Lq, Lk = L // CPB, L
    HP = H // 2                      # head pairs
    DC = D // P                      # contraction chunks for projections
    KC = Lk // P                     # key chunks (also v-proj l-chunks)
    QSW = min(512, Lq)               # q-slice width (matmul free dim)
    NQS = Lq // QSW                  # q slices
    NQB = QSW // P                   # 128-row q blocks per slice
    NKS = Lk // QSW
    HDK, HDV = H * DK, H * DV
    VW = DV + 1                      # per-head v width incl ones column
    f32 = mybir.dt.float32
    mm_dt = mybir.dt.float32r if cfg["MM_DT"] == "float32r" else f32
    Act = mybir.ActivationFunctionType
    Alu = mybir.AluOpType

    nc = bacc.Bacc("TRN2", target_bir_lowering=False, debug=False)

    def cast(ap):
        return ap.bitcast(mm_dt) if mm_dt is not f32 else ap

    def mm(out, lhsT, rhs, start, stop):
        nc.tensor.matmul(out, cast(lhsT), cast(rhs), start=start, stop=stop)

    with tile.TileContext(nc) as tc, ExitStack() as top:
        dram = top.enter_context(tc.tile_pool(name="dram", bufs=1, space="DRAM"))

        def din(name, shape):
            return dram.tile(shape, f32, kind="ExternalInput", name=name,
                             uniquify=False)

        def dout(name, shape):
            return dram.tile(shape, f32, kind="ExternalOutput", name=name,
                             uniquify=False)

        qT = din("qT", [D, Lq])
        qnat = din("qnat", [Lq, D])
        kT = din("kT", [D, Lk])
        vT = din("vT", [D, Lk])
        maskT = din("maskT", [Lk, Lq])
        wqT = din("wqT", [D, HDK])
        wkT = din("wkT", [D, HDK])
        wvT = din("wvT", [D, HDV])
        fcwT = din("fcwT", [HDV, D])
        smkh = din("smkh", [Lk, 1])
        smq = din("smq", [1, Lq])
        lng = din("lng", [1, D])
        lnb = din("lnb", [1, D])
        out_d = dout("out", [Lq, D])
        p_d = dout("p", [H, Lq, Lk])

        qhT_d = [dram.tile([P, Lq], f32, name=f"qhT_d{i}") for i in range(HP)]
        khT_d = [dram.tile([P, Lk], f32, name=f"khT_d{i}") for i in range(HP)]
        vhE_d = dram.tile([KC, P, H * VW], f32, name="vhE_d")
        oT_d = dram.tile([HP, P, Lq], f32, name="oT_d")

        # ---------------- Phase A: projections ----------------
        with ExitStack() as ph:
            wpool = ph.enter_context(tc.tile_pool(name="wpool", bufs=2))
            apool = ph.enter_context(tc.tile_pool(name="apool", bufs=DC + 2))
            astg = ph.enter_context(tc.tile_pool(name="astg", bufs=3))
            apsum = ph.enter_context(
                tc.tile_pool(name="apsum", bufs=3, space="PSUM"))
            vpsum = ph.enter_context(
                tc.tile_pool(name="vpsum", bufs=2, space="PSUM"))

            # --- v projection (natural layout + ones columns) ---
            wv_s = wpool.tile([P, DC, HDV], f32, tag="w")
            nc.sync.dma_start(out=wv_s,
                              in_=wvT[:].rearrange("(c p) n -> p c n", p=P))
            vts = []
            for d in range(DC):
                t = apool.tile([P, Lk], f32, tag="act")
                nc.sync.dma_start(out=t, in_=vT[d * P:(d + 1) * P, :])
                vts.append(t)
            for lc in range(KC):
                ps = vpsum.tile([P, HDV], f32, tag="psv")
                for d in range(DC):
                    for n0 in range(0, HDV, 512):
                        nn_ = min(512, HDV - n0)
                        mm(ps[:, n0:n0 + nn_],
                           vts[d][:, lc * P:(lc + 1) * P],
                           wv_s[:, d, n0:n0 + nn_],
                           start=(d == 0), stop=(d == DC - 1))
                st = astg.tile([P, H, VW], f32, tag="stgv")
                nc.scalar.copy(
                    st[:, :, 0:DV],
                    ps.rearrange("p (h v) -> p h v", h=H))
                nc.vector.memset(st[:, :, DV:VW], 1.0)
                nc.sync.dma_start(out=vhE_d[lc], in_=st.rearrange(
                    "p h v -> p (h v)"))

            # --- k projection: khT[hp] = wk chunk.T @ kT ---
            wk_s = wpool.tile([P, DC, HDK], f32, tag="w")
            nc.sync.dma_start(out=wk_s,
                              in_=wkT[:].rearrange("(c p) n -> p c n", p=P))
            kts = []
            for d in range(DC):
                t = apool.tile([P, Lk], f32, tag="act")
                nc.sync.dma_start(out=t, in_=kT[d * P:(d + 1) * P, :])
                kts.append(t)
            for ks in range(NKS):
                for hp in range(HP):
                    ps = apsum.tile([P, QSW], f32, tag="ps")
                    for d in range(DC):
                        mm(ps, wk_s[:, d, hp * P:(hp + 1) * P],
                           kts[d][:, ks * QSW:(ks + 1) * QSW],
                           start=(d == 0), stop=(d == DC - 1))
                    st = astg.tile([P, QSW], f32, tag="stg")
                    nc.scalar.copy(st, ps)
                    nc.sync.dma_start(
                        out=khT_d[hp][:, ks * QSW:(ks + 1) * QSW], in_=st)

            # --- q projection: qhT[hp] = (wq/8 chunk).T @ qT ---
            wq_s = wpool.tile([P, DC, HDK], f32, tag="w")
            nc.sync.dma_start(out=wq_s,
                              in_=wqT[:].rearrange("(c p) n -> p c n", p=P))
            qts = []
            for d in range(DC):
                t = apool.tile([P, Lq], f32, tag="act")
                nc.sync.dma_start(out=t, in_=qT[d * P:(d + 1) * P, :])
                qts.append(t)
            for qs in range(NQS):
                for hp in range(HP):
                    ps = apsum.tile([P, QSW], f32, tag="ps")
                    for d in range(DC):
                        mm(ps, wq_s[:, d, hp * P:(hp + 1) * P],
                           qts[d][:, qs * QSW:(qs + 1) * QSW],
                           start=(d == 0), stop=(d == DC - 1))
                    st = astg.tile([P, QSW], f32, tag="stg")
                    nc.scalar.copy(st, ps)
                    nc.sync.dma_start(
                        out=qhT_d[hp][:, qs * QSW:(qs + 1) * QSW], in_=st)

        # ---------------- Phase B: attention ----------------
        with ExitStack() as ph:
            assert KC % 2 == 0
            const = ph.enter_context(tc.tile_pool(name="const", bufs=1))
            mpool = ph.enter_context(tc.tile_pool(name="mpool", bufs=1))
            hppool = ph.enter_context(tc.tile_pool(name="hppool", bufs=2))
            epool = ph.enter_context(
                tc.tile_pool(name="epool", bufs=KC // 2 + 2))
            eipool = ph.enter_context(tc.tile_pool(name="eipool", bufs=2))
            prb = ph.enter_context(tc.tile_pool(name="prb", bufs=2))
            small = ph.enter_context(tc.tile_pool(name="small", bufs=1))
            otst = ph.enter_context(tc.tile_pool(name="otst", bufs=2))
            opsum = ph.enter_context(
                tc.tile_pool(name="opsum", bufs=2, space="PSUM"))
            tpsum = ph.enter_context(
                tc.tile_pool(name="tpsum", bufs=2, space="PSUM"))
            rpsum = ph.enter_context(
                tc.tile_pool(name="rpsum", bufs=1, space="PSUM"))
            spsum = ph.enter_context(
                tc.tile_pool(name="spsum", bufs=3, space="PSUM"))

            ident = const.tile([P, P], f32)
            from concourse.masks import make_identity
            make_identity(nc, ident)
            zeros_t = const.tile([P, 2, QSW], f32)
            nc.vector.memset(zeros_t, 0.0)
            ones64 = const.tile([1, DV], f32)
            nc.vector.memset(ones64, 1.0)
            one1 = const.tile([1, 1], f32)
            nc.vector.memset(one1, 1.0)

            # masked-position indicator H in uint8, [128, KC, Lq]
            h_u8 = mpool.tile([P, KC, Lq], mybir.dt.uint8)
            HBW = min(Lq, 512)
            with tc.tile_pool(name="hbuild", bufs=1) as hb:
                half_b = hb.tile([P, HBW], f32)
                nc.vector.memset(half_b, 0.5)
                for q0_ in range(0, Lq, HBW):
                    smq_b = hb.tile([P, HBW], f32, tag="smqb")
                    nc.sync.dma_start(
                        out=smq_b,
                        in_=bass.AP(tensor=smq[:].tensor,
                                    offset=smq[:].offset + q0_,
                                    ap=[[0, P], [1, HBW]]))
                    for kc in range(KC):
                        smk_t = small.tile([P, 1], f32, tag="smk")
                        nc.sync.dma_start(out=smk_t,
                                          in_=smkh[kc * P:(kc + 1) * P, :])
                        nc.vector.scalar_tensor_tensor(
                            out=h_u8[:, kc, q0_:q0_ + HBW], in0=smq_b,
                            scalar=smk_t, in1=half_b,
                            op0=Alu.mult, op1=Alu.add)

            mask_s = mpool.tile([P, KC, Lq], f32)
            nc.sync.dma_start(out=mask_s,
                              in_=maskT[:].rearrange("(c p) q -> p c q", p=P))

            TG = min(4, KC)  # transposes per eviction group
            TGW = TG * P
            for hp in range(HP):
                khT_hp = hppool.tile([P, Lk], f32, tag="kh")
                nc.sync.dma_start(out=khT_hp, in_=khT_d[hp][:])
                qhT_hp = hppool.tile([P, Lq], f32, tag="qh")
                nc.sync.dma_start(out=qhT_hp, in_=qhT_d[hp][:])
                vh_hp = hppool.tile([P, KC, 2 * VW], f32, tag="vh")
                nc.sync.dma_start(
                    out=vh_hp,
                    in_=vhE_d[:, :, 2 * VW * hp:2 * VW * (hp + 1)].rearrange(
                        "c p v -> p c v"))

                for h2 in range(2):
                    h = 2 * hp + h2
                    r0 = DK * h2  # partition row offset within pair tiles
                    for qs in range(NQS):
                        q0 = qs * QSW
                        psO = opsum.tile([DV + 1, QSW], f32, tag="psO")
                        es = []
                        for kc2 in range(KC // 2):
                            # two k-chunks pair into one [P, 2, QSW] SBUF
                            # tile so exp runs once per pair; mask-add +
                            # select stay per-chunk so PSUM frees fast.
                            ei = eipool.tile([P, 2, QSW], f32, tag="ei")
                            for j in range(2):
                                kc = 2 * kc2 + j
                                psS = spsum.tile([P, QSW], f32, tag="psS")
                                mm(psS,
                                   khT_hp[r0:r0 + DK, kc * P:(kc + 1) * P],
                                   qhT_hp[r0:r0 + DK, q0:q0 + QSW],
                                   start=True, stop=True)
                                nc.vector.tensor_tensor(
                                    ei[:, j, :], psS,
                                    mask_s[:, kc, q0:q0 + QSW], Alu.add)
                                nc.vector.copy_predicated(
                                    ei[:, j, :], h_u8[:, kc, q0:q0 + QSW],
                                    zeros_t[:, 0, :])
                            ee = epool.tile([P, 2, QSW], f32, tag="ee")
                            nc.scalar.activation(ee, ei, Act.Exp)
                            es.append(ee)
                            for j in range(2):
                                kc = 2 * kc2 + j
                                mm(psO, vh_hp[:, kc, VW * h2:VW * h2 + VW],
                                   ee[:, j, :],
                                   start=(kc == 0), stop=(kc == KC - 1))

                        # denominators -> reciprocal
                        dsb = small.tile([1, QSW], f32, tag="dsb")
                        nc.scalar.copy(dsb, psO[DV:DV + 1, :])
                        rsb = small.tile([1, QSW], f32, tag="rsb")
                        nc.vector.reciprocal(rsb, dsb)

                        # broadcast recip across partitions [DV, QSW]
                        psRB = rpsum.tile([DV, QSW], f32, tag="r")
                        mm(psRB, ones64, rsb, start=True, stop=True)
                        rbs = small.tile([DV, QSW], f32, tag="rbs")
                        nc.scalar.copy(rbs, psRB)

                        # normalized O^T spill
                        otn = otst.tile([DV, QSW], f32, tag="otn")
                        nc.vector.tensor_tensor(otn, psO[0:DV, :], rbs,
                                                Alu.mult)
                        nc.sync.dma_start(
                            out=oT_d[hp, r0:r0 + DK, q0:q0 + QSW], in_=otn)

                        # per-q-row recip columns [128, NQB]
                        rcol = small.tile([P, NQB], f32, tag="rcol")
                        for qb in range(NQB):
                            psRC = rpsum.tile([P, 1], f32, tag="r")
                            mm(psRC, rsb[:, qb * P:(qb + 1) * P],
                               one1, start=True, stop=True)
                            nc.scalar.copy(rcol[:, qb:qb + 1], psRC)

                        # P output: transpose E, normalize during eviction.
                        # TG transposes share one PSUM bank -> 1 ACT evict.
                        for qb in range(NQB):
                            pr = prb.tile([P, Lk], f32, tag="pr")
                            for kg in range(KC // TG):
                                psT = tpsum.tile([P, TGW], f32, tag="psT")
                                for j in range(TG):
                                    kc = kg * TG + j
                                    nc.tensor.transpose(
                                        psT[:, j * P:(j + 1) * P],
                                        es[kc // 2][:, kc % 2,
                                                    qb * P:(qb + 1) * P],
                                        ident)
                                nc.scalar.activation(
                                    pr[:, kg * TG * P:(kg + 1) * TG * P],
                                    psT, Act.Copy, scale=rcol[:, qb:qb + 1])
                            nc.sync.dma_start(
                                out=p_d[h, q0 + qb * P:q0 + (qb + 1) * P, :],
                                in_=pr)

        # ---------------- Phase C: fc + residual + LayerNorm ----------------
        with ExitStack() as ph:
            wpool = ph.enter_context(tc.tile_pool(name="wpool2", bufs=1))
            otpool = ph.enter_context(tc.tile_pool(name="otpool", bufs=HP))
            cact = ph.enter_context(tc.tile_pool(name="cact", bufs=3))
            cconst = ph.enter_context(tc.tile_pool(name="cconst", bufs=1))
            cpsum = ph.enter_context(
                tc.tile_pool(name="cpsum", bufs=2, space="PSUM"))
            csm = ph.enter_context(tc.tile_pool(name="csm", bufs=4))

            fcw_s = wpool.tile([P, HP, D], f32)
            nc.sync.dma_start(out=fcw_s,
                              in_=fcwT[:].rearrange("(c p) n -> p c n", p=P))
            ots = []
            for hp in range(HP):
                t = otpool.tile([P, Lq], f32, tag="ot")
                nc.sync.dma_start(out=t, in_=oT_d[hp])
                ots.append(t)
            lng_b = cconst.tile([P, D], f32)
            nc.sync.dma_start(
                out=lng_b, in_=bass.AP(tensor=lng[:].tensor,
                                       offset=lng[:].offset,
                                       ap=[[0, P], [1, D]]))
            lnb_b = cconst.tile([P, D], f32)
            nc.sync.dma_start(
                out=lnb_b, in_=bass.AP(tensor=lnb[:].tensor,
                                       offset=lnb[:].offset,
                                       ap=[[0, P], [1, D]]))
            eps_t = cconst.tile([P, 1], f32)
            nc.vector.memset(eps_t, EPS)

            NSG = (D + 511) // 512  # bn_stats subgroups
            SGW = D // NSG
            for lc in range(Lq // P):
                psF = cpsum.tile([P, D], f32, tag="psF")
                for hp in range(HP):
                    for n0 in range(0, D, 512):
                        nn_ = min(512, D - n0)
                        mm(psF[:, n0:n0 + nn_],
                           ots[hp][:, lc * P:(lc + 1) * P],
                           fcw_s[:, hp, n0:n0 + nn_],
                           start=(hp == 0), stop=(hp == HP - 1))
                qn = cact.tile([P, D], f32, tag="qn")
                nc.sync.dma_start(out=qn, in_=qnat[lc * P:(lc + 1) * P, :])
                tsb = cact.tile([P, D], f32, tag="tsb")
                nc.vector.tensor_tensor(tsb, psF, qn, Alu.add)

                stats = csm.tile([P, NSG, 6], f32, tag="stats")
                for sg in range(NSG):
                    nc.vector.bn_stats(out=stats[:, sg, :],
                                       in_=tsb[:, sg * SGW:(sg + 1) * SGW])
                mv = csm.tile([P, 2], f32, tag="mv")
                nc.vector.bn_aggr(out=mv, in_=stats)
                srt = csm.tile([P, 1], f32, tag="srt")
                nc.scalar.activation(srt, mv[:, 1:2], Act.Sqrt, bias=eps_t)
                rstd = csm.tile([P, 1], f32, tag="rstd")
                nc.vector.reciprocal(rstd, srt)
                xn = cact.tile([P, D], f32, tag="xn")
                nc.vector.tensor_scalar(out=xn, in0=tsb, scalar1=mv[:, 0:1],
                                        scalar2=rstd, op0=Alu.subtract,
                                        op1=Alu.mult)
                y1 = cact.tile([P, D], f32, tag="y1")
                nc.vector.tensor_tensor(y1, xn, lng_b, Alu.mult)
                yo = cact.tile([P, D], f32, tag="yo")
                nc.vector.tensor_tensor(yo, y1, lnb_b, Alu.add)
                nc.sync.dma_start(out=out_d[lc * P:(lc + 1) * P, :], in_=yo)

    nc.compile()
    return nc


def host_prep(cfg, q, k, v, mask, src_mask, wq, wk, wv, fc_w, ln_g, ln_b):
    """Build per-core in_maps."""
    B, L, D, H = cfg["B"], cfg["L"], cfg["D"], cfg["H"]
    DK = cfg["DK"]
    NC = cfg["NCORES"]
    CPB = NC // B
    Lq = L // CPB
    f32 = np.float32
    c_ = np.ascontiguousarray

    sm = np.where(src_mask == 0, -1.0, 1.0).astype(f32)  # [B, L]
    wqTs = c_((wq.astype(f32) / math.sqrt(DK)).T)        # [D, H*DK]
    wkTs = c_(wk.astype(f32).T)
    wvTs = c_(wv.astype(f32).T)
    fcwTs = c_(fc_w.astype(f32).T)                       # [H*DV, D]

    in_maps = []
    for c in range(NC):
        b, s = c // CPB, c % CPB
        sl = slice(s * Lq, (s + 1) * Lq)
        in_maps.append({
            "qT": c_(q[b, sl, :].T),
            "qnat": c_(q[b, sl, :]),
            "kT": c_(k[b].T),
            "vT": c_(v[b].T),
            "maskT": c_(mask[b, 0, sl, :].T),
            "wqT": wqTs, "wkT": wkTs, "wvT": wvTs, "fcwT": fcwTs,
            "smkh": c_((-0.5 * sm[b]).reshape(L, 1)),
            "smq": c_(sm[b, sl].reshape(1, Lq)),
            "lng": c_(ln_g.astype(f32).reshape(1, D)),
            "lnb": c_(ln_b.astype(f32).reshape(1, D)),
        })
    return in_maps


def assemble(cfg, results):
    B, L, D, H = cfg["B"], cfg["L"], cfg["D"], cfg["H"]
    NC = cfg["NCORES"]
    CPB = NC // B
    Lq = L // CPB
    out = np.empty((B, L, D), np.float32)
    p = np.empty((B, H, L, L), np.float32)
    for c in range(NC):
        b, s = c // CPB, c % CPB
        sl = slice(s * Lq, (s + 1) * Lq)
        out[b, sl, :] = results[c]["out"]
        p[b, :, sl, :] = results[c]["p"]
    return out, p


_NC_CACHE = {}


def _get_nc(cfg_key):
    if cfg_key not in _NC_CACHE:
        cfg = dict(CFG_FULL)
        _NC_CACHE[cfg_key] = build_bass(cfg)
    return _NC_CACHE[cfg_key]


def kernel(q, k, v, mask, src_mask, wq, wk, wv, fc_w, ln_g, ln_b):
    from concourse.bass_utils import run_bass_kernel_spmd
    cfg = dict(CFG_FULL)
    q = np.asarray(q, np.float32)
    k = np.asarray(k, np.float32)
    v = np.asarray(v, np.float32)
    mask = np.asarray(mask, np.float32)
    src_mask = np.asarray(src_mask)
    nc = _get_nc("full")
    in_maps = host_prep(cfg, q, k, v, mask, src_mask,
                        np.asarray(wq), np.asarray(wk), np.asarray(wv),
                        np.asarray(fc_w), np.asarray(ln_g), np.asarray(ln_b))
    res = run_bass_kernel_spmd(nc, in_maps, core_ids=list(range(cfg["NCORES"])))
    return assemble(cfg, res.results)


# revision 37
# speedup vs baseline: 2133.3046x; 2133.3046x over previous
"""Trainium2 Bass kernel for nn_MultiHeadAttention (B=4, L=2048, D=1024, H=16).

Sharding: each of 8 cores handles (batch b = core//2, q-row half = core%2):
Lq=1024 query rows, full Lk=2048 keys, all 16 heads. No collectives; K/V
projections are duplicated across the 2 cores sharing a batch.

Device pipeline per core:
  Phase A: QKV projections (q/k in head-transposed layout [dk, L], v in
           natural layout [L, dv] with an appended ones column per head for
           free softmax denominators), spilled to DRAM scratch.
  Phase B: per head: S^T = khT.T @ qhT (K=dk=64) -> +maskT (DVE) -> zero
           masked entries (copy_predicated, replicating the reference's
           -1e-9 fill through exp()==1.0) -> exp (ACT) -> O^T = vh.T @ E
           accumulated over k-chunks (65th row = denominators) -> P output
           via PE-transpose of E with normalization folded into the ACT
           PSUM eviction (scale = 1/Z per q row).
  Phase C: fc projection (contraction over heads), + residual, LayerNorm.
"""

import math
from contextlib import ExitStack

import numpy as np

import concourse.bass as bass
import concourse.mybir as mybir
import concourse.tile as tile
from concourse import bacc

P = 128  # partitions

CFG_FULL = dict(B=4, L=2048, D=1024, H=16, DK=64, DV=64, NCORES=8, EPS=1e-6,
                MM_DT="float32")


def build_bass(cfg):
    """Build the per-core Bass program. Returns (nc, names dict)."""
    B, L, D, H = cfg["B"], cfg["L"], cfg["D"], cfg["H"]
    DK, DV, EPS = cfg["DK"], cfg["DV"], cfg["EPS"]
    CPB = 2 * B // cfg["B"] // 2  # placeholder; cores per batch fixed at 2
    CPB = cfg["NCORES"] // B
    Lq, Lk = L // CPB, L
    HP = H // 2                      # head pairs
    DC = D // P                      # contraction chunks for projections
    KC = Lk // P                     # key chunks (also v-proj l-chunks)
    QSW = min(512, Lq)               # q-slice width (matmul free dim)
    NQS = Lq // QSW                  # q slices
    NQB = QSW // P                   # 128-row q blocks per slice
    NKS = Lk // QSW
    HDK, HDV = H * DK, H * DV
    VW = DV + 1                      # per-head v width incl ones column
    f32 = mybir.dt.float32
    mm_dt = mybir.dt.float32r if cfg["MM_DT"] == "float32r" else f32
    Act = mybir.ActivationFunctionType
    Alu = mybir.AluOpType

    nc = bacc.Bacc("TRN2", target_bir_lowering=False, debug=False)

    def cast(ap):
        return ap.bitcast(mm_dt) if mm_dt is not f32 else ap

    def mm(out, lhsT, rhs, start, stop):
        nc.tensor.matmul(out, cast(lhsT), cast(rhs), start=start, stop=stop)

    with tile.TileContext(nc) as tc, ExitStack() as top:
        dram = top.enter_context(tc.tile_pool(name="dram", bufs=1, space="DRAM"))

        def din(name, shape):
            return dram.tile(shape, f32, kind="ExternalInput", name=name,
                             uniquify=False)

        def dout(name, shape):
            return dram.tile(shape, f32, kind="ExternalOutput", name=name,
                             uniquify=False)

        qT = din("qT", [D, Lq])
        qnat = din("qnat", [Lq, D])
        kT = din("kT", [D, Lk])
        vT = din("vT", [D, Lk])
        maskT = din("maskT", [Lk, Lq])
        wqT = din("wqT", [D, HDK])
        wkT = din("wkT", [D, HDK])
        wvT = din("wvT", [D, HDV])
        fcwT = din("fcwT", [HDV, D])
        smkh = din("smkh", [Lk, 1])
        smq = din("smq", [1, Lq])
        lng = din("lng", [1, D])
        lnb = din("lnb", [1, D])
        out_d = dout("out", [Lq, D])
        p_d = dout("p", [H, Lq, Lk])

        qhT_d = [dram.tile([P, Lq], f32, name=f"qhT_d{i}") for i in range(HP)]
        khT_d = [dram.tile([P, Lk], f32, name=f"khT_d{i}") for i in range(HP)]
        vhE_d = dram.tile([KC, P, H * VW], f32, name="vhE_d")
        oT_d = dram.tile([HP, P, Lq], f32, name="oT_d")

        # ---------------- Phase A: projections ----------------
        with ExitStack() as ph:
            wpool = ph.enter_context(tc.tile_pool(name="wpool", bufs=2))
            apool = ph.enter_context(tc.tile_pool(name="apool", bufs=DC + 2))
            astg = ph.enter_context(tc.tile_pool(name="astg", bufs=3))
            apsum = ph.enter_context(
                tc.tile_pool(name="apsum", bufs=3, space="PSUM"))
            vpsum = ph.enter_context(
                tc.tile_pool(name="vpsum", bufs=2, space="PSUM"))

            # --- v projection (natural layout + ones columns) ---
            wv_s = wpool.tile([P, DC, HDV], f32, tag="w")
            nc.sync.dma_start(out=wv_s,
                              in_=wvT[:].rearrange("(c p) n -> p c n", p=P))
            vts = []
            for d in range(DC):
                t = apool.tile([P, Lk], f32, tag="act")
                nc.sync.dma_start(out=t, in_=vT[d * P:(d + 1) * P, :])
                vts.append(t)
            for lc in range(KC):
                ps = vpsum.tile([P, HDV], f32, tag="psv")
                for d in range(DC):
                    for n0 in range(0, HDV, 512):
                        nn_ = min(512, HDV - n0)
                        mm(ps[:, n0:n0 + nn_],
                           vts[d][:, lc * P:(lc + 1) * P],
                           wv_s[:, d, n0:n0 + nn_],
                           start=(d == 0), stop=(d == DC - 1))
                st = astg.tile([P, H, VW], f32, tag="stgv")
                nc.scalar.copy(
                    st[:, :, 0:DV],
                    ps.rearrange("p (h v) -> p h v", h=H))
                nc.vector.memset(st[:, :, DV:VW], 1.0)
                nc.sync.dma_start(out=vhE_d[lc], in_=st.rearrange(
                    "p h v -> p (h v)"))

            # --- k projection: khT[hp] = wk chunk.T @ kT ---
            wk_s = wpool.tile([P, DC, HDK], f32, tag="w")
            nc.sync.dma_start(out=wk_s,
                              in_=wkT[:].rearrange("(c p) n -> p c n", p=P))
            kts = []
            for d in range(DC):
                t = apool.tile([P, Lk], f32, tag="act")
                nc.sync.dma_start(out=t, in_=kT[d * P:(d + 1) * P, :])
                kts.append(t)
            for ks in range(NKS):
                for hp in range(HP):
                    ps = apsum.tile([P, QSW], f32, tag="ps")
                    for d in range(DC):
                        mm(ps, wk_s[:, d, hp * P:(hp + 1) * P],
                           kts[d][:, ks * QSW:(ks + 1) * QSW],
                           start=(d == 0), stop=(d == DC - 1))
                    st = astg.tile([P, QSW], f32, tag="stg")
                    nc.scalar.copy(st, ps)
                    nc.sync.dma_start(
                        out=khT_d[hp][:, ks * QSW:(ks + 1) * QSW], in_=st)

            # --- q projection: qhT[hp] = (wq/8 chunk).T @ qT ---
            wq_s = wpool.tile([P, DC, HDK], f32, tag="w")
            nc.sync.dma_start(out=wq_s,
                              in_=wqT[:].rearrange("(c p) n -> p c n", p=P))
            qts = []
            for d in range(DC):
                t = apool.tile([P, Lq], f32, tag="act")
                nc.sync.dma_start(out=t, in_=qT[d * P:(d + 1) * P, :])
                qts.append(t)
            for qs in range(NQS):
                for hp in range(HP):
                    ps = apsum.tile([P, QSW], f32, tag="ps")
                    for d in range(DC):
                        mm(ps, wq_s[:, d, hp * P:(hp + 1) * P],
                           qts[d][:, qs * QSW:(qs + 1) * QSW],
                           start=(d == 0), stop=(d == DC - 1))
                    st = astg.tile([P, QSW], f32, tag="stg")
                    nc.scalar.copy(st, ps)
                    nc.sync.dma_start(
                        out=qhT_d[hp][:, qs * QSW:(qs + 1) * QSW], in_=st)

        # ---------------- Phase B: attention ----------------
        with ExitStack() as ph:
            assert KC % 2 == 0
            const = ph.enter_context(tc.tile_pool(name="const", bufs=1))
            mpool = ph.enter_context(tc.tile_pool(name="mpool", bufs=1))
            opsum = ph.enter_context(
                tc.tile_pool(name="opsum", bufs=2, space="PSUM"))
            tpsum = ph.enter_context(
                tc.tile_pool(name="tpsum", bufs=2, space="PSUM"))
            rpsum = ph.enter_context(
                tc.tile_pool(name="rpsum", bufs=1, space="PSUM"))
            spsum = ph.enter_context(
                tc.tile_pool(name="spsum", bufs=3, space="PSUM"))

            ident = const.tile([P, P], f32)
            from concourse.masks import make_identity
            make_identity(nc, ident)
            zeros_t = const.tile([P, QSW], f32)
            nc.vector.memset(zeros_t, 0.0)
            ones64 = const.tile([1, DV], f32)
            nc.vector.memset(ones64, 1.0)
            one1 = const.tile([1, 1], f32)
            nc.vector.memset(one1, 1.0)

            # masked-position indicator H in uint8, [128, KC, Lq]
            h_u8 = mpool.tile([P, KC, Lq], mybir.dt.uint8)
            HBW = min(Lq, 512)
            with tc.tile_pool(name="hbuild", bufs=1) as hb:
                half_b = hb.tile([P, HBW], f32)
                nc.vector.memset(half_b, 0.5)
                for q0_ in range(0, Lq, HBW):
                    smq_b = hb.tile([P, HBW], f32, tag="smqb")
                    nc.sync.dma_start(
                        out=smq_b,
                        in_=bass.AP(tensor=smq[:].tensor,
                                    offset=smq[:].offset + q0_,
                                    ap=[[0, P], [1, HBW]]))
                    for kc in range(KC):
                        smk_t = hb.tile([P, 1], f32, tag="smk")
                        nc.sync.dma_start(out=smk_t,
                                          in_=smkh[kc * P:(kc + 1) * P, :])
                        nc.vector.scalar_tensor_tensor(
                            out=h_u8[:, kc, q0_:q0_ + HBW], in0=smq_b,
                            scalar=smk_t, in1=half_b,
                            op0=Alu.mult, op1=Alu.add)

            mask_s = mpool.tile([P, KC, Lq], f32)
            nc.sync.dma_start(out=mask_s,
                              in_=maskT[:].rearrange("(c p) q -> p c q", p=P))

            hppool = ph.enter_context(tc.tile_pool(name="hppool", bufs=2))
            epool = ph.enter_context(
                tc.tile_pool(name="epool", bufs=KC // 2 + 7))
            eipool = ph.enter_context(tc.tile_pool(name="eipool", bufs=2))
            prb = ph.enter_context(tc.tile_pool(name="prb", bufs=3))
            small = ph.enter_context(tc.tile_pool(name="small", bufs=1))
            otst = ph.enter_context(tc.tile_pool(name="otst", bufs=2))

            TG = min(4, KC)  # transposes per eviction group
            TGW = TG * P
            for hp in range(HP):
                khT_hp = hppool.tile([P, Lk], f32, tag="kh")
                nc.sync.dma_start(out=khT_hp, in_=khT_d[hp][:])
                qhT_hp = hppool.tile([P, Lq], f32, tag="qh")
                nc.sync.dma_start(out=qhT_hp, in_=qhT_d[hp][:])
                vh_hp = hppool.tile([P, KC, 2 * VW], f32, tag="vh")
                nc.sync.dma_start(
                    out=vh_hp,
                    in_=vhE_d[:, :, 2 * VW * hp:2 * VW * (hp + 1)].rearrange(
                        "c p v -> p c v"))

                for h2 in range(2):
                    h = 2 * hp + h2
                    r0 = DK * h2  # partition row offset within pair tiles
                    for qs in range(NQS):
                        q0 = qs * QSW
                        psO = opsum.tile([DV + 1, QSW], f32, tag="psO")
                        es = []
                        for kc2 in range(KC // 2):
                            # two k-chunks pair into one [P, 2, QSW] SBUF
                            # tile so exp runs once per pair; mask-add +
                            # select stay per-chunk so PSUM frees fast.
                            ei = eipool.tile([P, 2, QSW], f32, tag="ei")
                            for j in range(2):
                                kc = 2 * kc2 + j
                                psS = spsum.tile([P, QSW], f32, tag="psS")
                                mm(psS,
                                   khT_hp[r0:r0 + DK, kc * P:(kc + 1) * P],
                                   qhT_hp[r0:r0 + DK, q0:q0 + QSW],
                                   start=True, stop=True)
                                nc.vector.tensor_tensor(
                                    ei[:, j, :], psS,
                                    mask_s[:, kc, q0:q0 + QSW], Alu.add)
                                nc.vector.copy_predicated(
                                    ei[:, j, :], h_u8[:, kc, q0:q0 + QSW],
                                    zeros_t)
                            ee = epool.tile([P, 2, QSW], f32, tag="ee")
                            nc.scalar.activation(ee, ei, Act.Exp)
                            es.append(ee)
                            for j in range(2):
                                kc = 2 * kc2 + j
                                mm(psO, vh_hp[:, kc, VW * h2:VW * h2 + VW],
                                   ee[:, j, :],
                                   start=(kc == 0), stop=(kc == KC - 1))

                        # denominators -> reciprocal (DVE reads PSUM)
                        rsb = small.tile([1, QSW], f32, tag="rsb")
                        nc.vector.reciprocal(rsb, psO[DV:DV + 1, :])

                        # broadcast recip across partitions [DV, QSW]
                        psRB = rpsum.tile([DV, QSW], f32, tag="r")
                        mm(psRB, ones64, rsb, start=True, stop=True)
                        rbs = small.tile([DV, QSW], f32, tag="rbs")
                        nc.scalar.copy(rbs, psRB)

                        # normalized O^T spill
                        otn = otst.tile([DV, QSW], f32, tag="otn")
                        nc.vector.tensor_tensor(otn, psO[0:DV, :], rbs,
                                                Alu.mult)
                        nc.sync.dma_start(
                            out=oT_d[hp, r0:r0 + DK, q0:q0 + QSW], in_=otn)

                        # per-q-row recip columns [128, NQB]
                        rcol = small.tile([P, NQB], f32, tag="rcol")
                        for qb in range(NQB):
                            psRC = rpsum.tile([P, 1], f32, tag="r")
                            mm(psRC, rsb[:, qb * P:(qb + 1) * P],
                               one1, start=True, stop=True)
                            nc.scalar.copy(rcol[:, qb:qb + 1], psRC)

                        # P output: transpose E, normalize during eviction.
                        # TG transposes share one PSUM bank -> 1 ACT evict
                        # -> immediate DMA of the [P, TGW] strip.
                        for qb in range(NQB):
                            for kg in range(KC // TG):
                                psT = tpsum.tile([P, TGW], f32, tag="psT")
                                for j in range(TG):
                                    kc = kg * TG + j
                                    nc.tensor.transpose(
                                        psT[:, j * P:(j + 1) * P],
                                        es[kc // 2][:, kc % 2,
                                                    qb * P:(qb + 1) * P],
                                        ident)
                                pr = prb.tile([P, TGW], f32, tag="pr")
                                nc.scalar.activation(
                                    pr, psT, Act.Copy,
                                    scale=rcol[:, qb:qb + 1])
                                nc.sync.dma_start(
                                    out=p_d[h, q0 + qb * P:q0 + (qb + 1) * P,
                                            kg * TGW:(kg + 1) * TGW],
                                    in_=pr)

        # ---------------- Phase C: fc + residual + LayerNorm ----------------
        with ExitStack() as ph:
            wpool = ph.enter_context(tc.tile_pool(name="wpool2", bufs=1))
            otpool = ph.enter_context(tc.tile_pool(name="otpool", bufs=HP))
            cact = ph.enter_context(tc.tile_pool(name="cact", bufs=3))
            cconst = ph.enter_context(tc.tile_pool(name="cconst", bufs=1))
            cpsum = ph.enter_context(
                tc.tile_pool(name="cpsum", bufs=2, space="PSUM"))
            csm = ph.enter_context(tc.tile_pool(name="csm", bufs=4))

            fcw_s = wpool.tile([P, HP, D], f32)
            nc.sync.dma_start(out=fcw_s,
                              in_=fcwT[:].rearrange("(c p) n -> p c n", p=P))
            ots = []
            for hp in range(HP):
                t = otpool.tile([P, Lq], f32, tag="ot")
                nc.sync.dma_start(out=t, in_=oT_d[hp])
                ots.append(t)
            lng_b = cconst.tile([P, D], f32)
            nc.sync.dma_start(
                out=lng_b, in_=bass.AP(tensor=lng[:].tensor,
                                       offset=lng[:].offset,
                                       ap=[[0, P], [1, D]]))
            lnb_b = cconst.tile([P, D], f32)
            nc.sync.dma_start(
                out=lnb_b, in_=bass.AP(tensor=lnb[:].tensor,
                                       offset=lnb[:].offset,
                                       ap=[[0, P], [1, D]]))
            eps_t = cconst.tile([P, 1], f32)
            nc.vector.memset(eps_t, EPS)

            NSG = (D + 511) // 512  # bn_stats subgroups
            SGW = D // NSG
            for lc in range(Lq // P):
                psF = cpsum.tile([P, D], f32, tag="psF")
                for hp in range(HP):
                    for n0 in range(0, D, 512):
                        nn_ = min(512, D - n0)
                        mm(psF[:, n0:n0 + nn_],
                           ots[hp][:, lc * P:(lc + 1) * P],
                           fcw_s[:, hp, n0:n0 + nn_],
                           start=(hp == 0), stop=(hp == HP - 1))
                qn = cact.tile([P, D], f32, tag="qn")
                nc.sync.dma_start(out=qn, in_=qnat[lc * P:(lc + 1) * P, :])
                tsb = cact.tile([P, D], f32, tag="tsb")
                nc.vector.tensor_tensor(tsb, psF, qn, Alu.add)

                stats = csm.tile([P, NSG, 6], f32, tag="stats")
                for sg in range(NSG):
                    nc.vector.bn_stats(out=stats[:, sg, :],
                                       in_=tsb[:, sg * SGW:(sg + 1) * SGW])
                mv = csm.tile([P, 2], f32, tag="mv")
                nc.vector.bn_aggr(out=mv, in_=stats)
                srt = csm.tile([P, 1], f32, tag="srt")
                nc.scalar.activation(srt, mv[:, 1:2], Act.Sqrt, bias=eps_t)
                rstd = csm.tile([P, 1], f32, tag="rstd")
                nc.vector.reciprocal(rstd, srt)
                xn = cact.tile([P, D], f32, tag="xn")
                nc.vector.tensor_scalar(out=xn, in0=tsb, scalar1=mv[:, 0:1],
                                        scalar2=rstd, op0=Alu.subtract,
                                        op1=Alu.mult)
                y1 = cact.tile([P, D], f32, tag="y1")
                nc.vector.tensor_tensor(y1, xn, lng_b, Alu.mult)
                yo = cact.tile([P, D], f32, tag="yo")
                nc.vector.tensor_tensor(yo, y1, lnb_b, Alu.add)
                nc.sync.dma_start(out=out_d[lc * P:(lc + 1) * P, :], in_=yo)

    nc.compile()
    return nc


def host_prep(cfg, q, k, v, mask, src_mask, wq, wk, wv, fc_w, ln_g, ln_b):
    """Build per-core in_maps."""
    B, L, D, H = cfg["B"], cfg["L"], cfg["D"], cfg["H"]
    DK = cfg["DK"]
    NC = cfg["NCORES"]
    CPB = NC // B
    Lq = L // CPB
    f32 = np.float32
    c_ = np.ascontiguousarray

    sm = np.where(src_mask == 0, -1.0, 1.0).astype(f32)  # [B, L]
    wqTs = c_((wq.astype(f32) / math.sqrt(DK)).T)        # [D, H*DK]
    wkTs = c_(wk.astype(f32).T)
    wvTs = c_(wv.astype(f32).T)
    fcwTs = c_(fc_w.astype(f32).T)                       # [H*DV, D]

    in_maps = []
    for c in range(NC):
        b, s = c // CPB, c % CPB
        sl = slice(s * Lq, (s + 1) * Lq)
        in_maps.append({
            "qT": c_(q[b, sl, :].T),
            "qnat": c_(q[b, sl, :]),
            "kT": c_(k[b].T),
            "vT": c_(v[b].T),
            "maskT": c_(mask[b, 0, sl, :].T),
            "wqT": wqTs, "wkT": wkTs, "wvT": wvTs, "fcwT": fcwTs,
            "smkh": c_((-0.5 * sm[b]).reshape(L, 1)),
            "smq": c_(sm[b, sl].reshape(1, Lq)),
            "lng": c_(ln_g.astype(f32).reshape(1, D)),
            "lnb": c_(ln_b.astype(f32).reshape(1, D)),
        })
    return in_maps


def assemble(cfg, results):
    B, L, D, H = cfg["B"], cfg["L"], cfg["D"], cfg["H"]
    NC = cfg["NCORES"]
    CPB = NC // B
    Lq = L // CPB
    out = np.empty((B, L, D), np.float32)
    p = np.empty((B, H, L, L), np.float32)
    for c in range(NC):
        b, s = c // CPB, c % CPB
        sl = slice(s * Lq, (s + 1) * Lq)
        out[b, sl, :] = results[c]["out"]
        p[b, :, sl, :] = results[c]["p"]
    return out, p


_NC_CACHE = {}


def _get_nc(cfg_key):
    if cfg_key not in _NC_CACHE:
        cfg = dict(CFG_FULL)
        _NC_CACHE[cfg_key] = build_bass(cfg)
    return _NC_CACHE[cfg_key]


def kernel(q, k, v, mask, src_mask, wq, wk, wv, fc_w, ln_g, ln_b):
    from concourse.bass_utils import run_bass_kernel_spmd
    cfg = dict(CFG_FULL)
    q = np.asarray(q, np.float32)
    k = np.asarray(k, np.float32)
    v = np.asarray(v, np.float32)
    mask = np.asarray(mask, np.float32)
    src_mask = np.asarray(src_mask)
    nc = _get_nc("full")
    in_maps = host_prep(cfg, q, k, v, mask, src_mask,
                        np.asarray(wq), np.asarray(wk), np.asarray(wv),
                        np.asarray(fc_w), np.asarray(ln_g), np.asarray(ln_b))
    res = run_bass_kernel_spmd(nc, in_maps, core_ids=list(range(cfg["NCORES"])))
    return assemble(cfg, res.results)
